# revision 50
# baseline (speedup 1.0000x reference)
"""Multi-head attention (B=2, S=2048, H=1024, NH=16 heads of 64) on 8 trn2
NeuronCores, tensor-parallel over heads with batch parallelism on top.

Sharding: core c handles batch b=c//4 and head-group g=c%4 (4 heads, 256 of
the 1024 hidden cols). Each core computes the partial output
ctx_g @ Wo[g_rows, :]; the host sums the 4 partials per batch and adds the
closed-form bias terms (bv @ Wo + bo; bq/bk are applied on-device).

Device math (per core), fp16 matmul operands, fp32 PSUM accumulation:

  qT/kT = Wq_g^T x_b^T (+bias/partition)  [2 head-pair tiles of 128 x 2048]
  v     = x_b Wv_g                        [16 tiles of 128 x (4*65)]; col 64
                                          of each head block = 1.0 (softmax
                                          denominator comes out of the ctx
                                          matmul for free)
  scoresT[k,q] = kT.T qT                  (PE; head pairs packed as
                                           [h0 512q | h1 512q] per 2-bank
                                           PSUM tile)
  expT = exp(0.125*scoresT + mask[k])     (one ACT op: scale+mask+exp; no max
                                           subtraction needed -- |scores|<~4)
  ctx[q,c] += expT_chunk^T v_aug          (PE; exp chunk [128k,128q] is the
                                           STATIONARY operand, v_aug [128k,65]
                                           moves -> 65 cycles/tile in the cost
                                           model instead of 512, and ctx lands
                                           in [q,c] orientation with D[q] in
                                           col 64 as a per-partition column)
  normalize: reciprocal of ctx[:,:,64] + per-partition tensor_scalar_mul
             straight into asm[q,c] (no transposes, no PSUM->SBUF ctx copy)
  out[q,:] = asm_n @ Wo_g                 (PE, via one transpose back to [c,q])

Precision: scores run fp8e4 (q/k cast+regrouped by gpsimd DMAs into the
DoubleRow 2x32 split-contraction layout, 0.5 cycles/row on the PE) except for
the first combo, which reads the f16 projections directly so the regroup
latency stays off the startup critical path. Everything else is fp16.

Schedule: one flat stream of 128 (combo, kc) score+exp steps keeps ACT (the
exp engine, ~137us busy, the bottleneck) fed continuously; ctx matmuls trail
~10 steps behind via a pop queue with a 6-step breathing gap at each combo
boundary (the previous combo's DVE normalize must free its 2 PSUM ctx banks
before the next combo's first ctx write). Projections, fp8 regroups and
output-projection tails are deadline-placed PE/Pool fillers. Combo order is
hp0-first to minimize projection front-load. Cost model: ~168us (PE 117us,
ACT 140us busy).
"""

import os
import sys

sys.path.insert(0, "/opt/trn_rl_repo")

import numpy as np

B, S, H, NH, HD = 2, 2048, 1024, 16, 64
NCORES = 8
HPC = 4          # heads per core
COLS = HPC * HD  # 256
KC = S // 128    # 16 k chunks
QB = 1024        # q block width (2 heads x 512 q)
NQT = S // 128   # 16 global q tiles
SC = 512         # seq chunk for projections

_CACHE = {}


def _build():
    import concourse.mybir as mybir
    import concourse.tile as tile
    from concourse import bacc
    from concourse.masks import make_identity

    f32 = mybir.dt.float32
    f16 = mybir.dt.float16
    f8 = mybir.dt.float8e4
    DR = mybir.MatmulPerfMode.DoubleRow
    Exp = mybir.ActivationFunctionType.Exp

    nc = bacc.Bacc("TRN2", target_bir_lowering=False, debug=False,
                   num_devices=NCORES)

    xT_d = nc.dram_tensor("xT", [H, S], f16, kind="ExternalInput").ap()
    wq_d = nc.dram_tensor("wq", [H, COLS], f16, kind="ExternalInput").ap()
    wk_d = nc.dram_tensor("wk", [H, COLS], f16, kind="ExternalInput").ap()
    wv_d = nc.dram_tensor("wv", [H, COLS], f16, kind="ExternalInput").ap()
    wo_d = nc.dram_tensor("wo", [COLS, H], f16, kind="ExternalInput").ap()
    bq_d = nc.dram_tensor("bq", [COLS], f32, kind="ExternalInput").ap()
    bk_d = nc.dram_tensor("bk", [COLS], f32, kind="ExternalInput").ap()
    mask_d = nc.dram_tensor("mask", [S], f32, kind="ExternalInput").ap()
    out_d = nc.dram_tensor("out", [S, H], f32, kind="ExternalOutput").ap()

    with tile.TileContext(nc) as tc:
        pers = tc.alloc_tile_pool(name="pers", bufs=1)
        # PSUM budget (8 banks of 2KB):
        #   psA: scores [128,1024] f32 (2 banks) x2 bufs           = 4
        #   psC: ctx accumulators [128,4,65] f32 (1 bank) x2 bufs  = 2
        #   psD: op [128,512] f32 (1) + t2p [128,256] f16 (1)      = 2
        psA = tc.alloc_tile_pool(name="psA", bufs=2, space="PSUM")
        psC = tc.alloc_tile_pool(name="psC", bufs=2, space="PSUM")
        psD = tc.alloc_tile_pool(name="psD", bufs=1, space="PSUM")
        work = tc.alloc_tile_pool(name="work", bufs=3)

        # q/k projected to f16; gpsimd cast-regroup DMAs produce the fp8
        # DoubleRow layout (head j's 64 d-dims as 2 k-tiles x 32 partitions)
        # for every combo except the first, whose scores run fp16 directly
        # so the regroup latency stays off the startup critical path.
        qT = [pers.tile([128, S], f16, tag=f"qT{i}", name=f"qT{i}")
              for i in range(2)]
        kT = [pers.tile([128, S], f16, tag=f"kT{i}", name=f"kT{i}")
              for i in range(2)]
        q8r = [pers.tile([32, 4, S], f8, tag=f"q8r{i}", name=f"q8r{i}")
               for i in range(2)]
        k8r = [pers.tile([32, 4, S], f8, tag=f"k8r{i}", name=f"k8r{i}")
               for i in range(2)]
        vt = [pers.tile([128, HPC * 65], f16, tag=f"v{i}", name=f"v{i}")
              for i in range(KC)]
        asm = [pers.tile([128, COLS], f16, tag=f"asm{i}", name=f"asm{i}")
               for i in range(NQT)]
        xt4 = [pers.tile([128, 2 * S], f16, tag=f"xt4{i}", name=f"xt4{i}")
               for i in range(4)]
        wq_a = pers.tile([128, 2048], f16, tag="wq", name="wq_a")
        wk_a = pers.tile([128, 2048], f16, tag="wk", name="wk_a")
        wv_a = pers.tile([128, 2048], f16, tag="wv", name="wv_a")
        wo_a = pers.tile([128, 2048], f16, tag="wo", name="wo_a")

        def xT(hc):
            """View of H-chunk hc of x^T: [128, S] slice of a packed tile."""
            return xt4[hc // 2][:, (hc % 2) * S:(hc % 2) * S + S]
        bq_s = pers.tile([128, 2], f32, tag="bq", name="bq_s")
        bk_s = pers.tile([128, 2], f32, tag="bk", name="bk_s")
        mask_s = pers.tile([128, KC], f32, tag="mask", name="mask_s")
        id128 = pers.tile([128, 128], f16, tag="id128", name="id128")
        z260 = pers.tile([128, 260], f16, tag="z260", name="z260")

        warm = pers.tile([1, 1], f32, tag="warm", name="warm")
        nc.gpsimd.memset(warm[:], 0.0)
        nc.scalar.activation(warm[:], warm[:], Exp)
        make_identity(nc, id128[:])
        nc.vector.memset(z260[:], 0.0)

        # Few large DMAs on one HWDGE queue (dispatch is ~650ns/DMA, so
        # batch aggressively), ordered so the projection pipeline starts as
        # early as possible (queue order = arrival order).
        def xt4_pair(t, lo, hi):
            out = xt4[t].rearrange("p (c s) -> p c s", c=2)[:, :, lo:hi]
            in_ = xT_d[t * 256:(t + 1) * 256, lo:hi].rearrange(
                "(c p) s -> p c s", p=128)
            nc.sync.dma_start(out, in_)

        def w_half(w_a, w_d, lo):
            nc.sync.dma_start(
                w_a.rearrange("p (c n) -> p c n", c=8)[:, 4 * lo:4 * lo + 4],
                w_d[512 * lo:512 * lo + 512].rearrange(
                    "(c p) n -> p c n", p=128))

        w_half(wq_a, wq_d, 0)
        for t in range(4):
            xt4_pair(t, 0, SC)
        w_half(wq_a, wq_d, 1)
        w_half(wk_a, wk_d, 0)
        w_half(wk_a, wk_d, 1)
        nc.sync.dma_start(bq_s[:], bq_d.rearrange("(a p) -> p a", p=128))
        nc.sync.dma_start(bk_s[:], bk_d.rearrange("(a p) -> p a", p=128))
        nc.sync.dma_start(mask_s[:], mask_d.rearrange("(a p) -> p a", p=128))
        for t in range(4):
            xt4_pair(t, SC, 2 * SC)
        nc.sync.dma_start(wv_a.rearrange("p (c n) -> p c n", c=8),
                          wv_d.rearrange("(c p) n -> p c n", p=128))
        for t in range(4):
            xt4_pair(t, 2 * SC, 3 * SC)
        for t in range(4):
            xt4_pair(t, 3 * SC, S)
        nc.sync.dma_start(wo_a.rearrange("p (c n) -> p c n", c=2),
                          wo_d.rearrange("(c p) n -> p c n", p=128))

        Ident = mybir.ActivationFunctionType.Identity

        def qk_proj(w_a, b_s, dst, r8, pi, sc, act=False, regroup=True,
                    hcs=range(8), ps=None):
            # psD slot (not psA): projections must not perturb the scores
            # double-buffer rotation, or ACT stalls an extra exp every time.
            if ps is None:
                ps = psD.tile([128, SC], f32, tag="op", name="pps")
            for hc in hcs:
                nc.tensor.matmul(
                    ps[:], w_a[:, hc * COLS + pi * 128:hc * COLS + pi * 128 + 128],
                    xT(hc)[:, sc * SC:(sc + 1) * SC],
                    start=(hc == 0), stop=(hc == 7))
            if hcs[-1] != 7:
                return ps
            if act:
                # phase-1: ACT has idle slots there
                nc.scalar.activation(dst[pi][:, sc * SC:(sc + 1) * SC], ps[:],
                                     Ident, bias=b_s[:, pi:pi + 1])
            else:
                nc.vector.tensor_scalar_add(dst[pi][:, sc * SC:(sc + 1) * SC],
                                            ps[:], b_s[:, pi:pi + 1])
            # cast-regroup this 512-q chunk into the fp8 DoubleRow
            # (p, (j,t)) layout; a DMA source AP cannot stride across
            # partitions in a free dim, so one plain partition-range DMA per
            # 32-partition group. Via GPSIMD software DGE (casts f16->f8;
            # waiting DMAs on the SP/ACT HWDGE rings would block those
            # engines' in-order sequencers).
            if regroup:
                for g in range(4):
                    nc.gpsimd.dma_start(
                        r8[pi][:, g, sc * SC:(sc + 1) * SC],
                        dst[pi][g * 32:(g + 1) * 32, sc * SC:(sc + 1) * SC])

        def v_proj(st):
            ps = psD.tile([128, COLS], f32, tag="op", name="vps")
            for hc in range(8):
                nc.tensor.matmul(ps[:], xT(hc)[:, st * 128:(st + 1) * 128],
                                 wv_a[:, hc * COLS:(hc + 1) * COLS],
                                 start=(hc == 0), stop=(hc == 7))
            nc.vector.memset(vt[st][:], 1.0)
            nc.vector.tensor_copy(
                vt[st].rearrange("p (h c) -> p h c", c=65)[:, :, 0:64],
                ps[:].rearrange("p (h c) -> p h c", c=64))

        # ---- attention machinery ----
        combos = [(0, 0), (0, 1), (0, 2), (0, 3),
                  (1, 0), (1, 1), (1, 2), (1, 3)]
        ctx_open = {}   # combo idx -> [h0_tile, h1_tile] PSUM accumulators
        pend = []       # FIFO of (combo_idx, kc, ex) awaiting ctx emission

        def emit_scores_exp(ci, kc):
            """scores+exp for one (combo, kc): 2 score matmuls into a
            double-buffered PSUM tile [128, 1024] = [h0 512q | h1 512q],
            one ACT exp op. The psA rotation self-throttles PE ~2 steps
            behind ACT."""
            hp, qb4 = combos[ci]
            qs = qb4 * 512
            sc_ps = psA.tile([128, QB], f32, tag="sc", name="sc_ps")
            for j in range(2):
                if ci == 0:
                    # first combo: fp16 from the f16 tiles directly -- its
                    # cast-regroups would otherwise gate the score stream
                    # during startup
                    nc.tensor.matmul(
                        sc_ps[:, j * 512:(j + 1) * 512],
                        kT[hp][j * 64:j * 64 + 64, kc * 128:(kc + 1) * 128],
                        qT[hp][j * 64:j * 64 + 64, qs:qs + 512],
                        start=True, stop=True)
                else:
                    nc.tensor.matmul(
                        sc_ps[:, j * 512:(j + 1) * 512],
                        k8r[hp][:, 2 * j:2 * j + 2, kc * 128:(kc + 1) * 128],
                        q8r[hp][:, 2 * j:2 * j + 2, qs:qs + 512],
                        start=True, stop=True, perf_mode=DR)
            ex = work.tile([128, QB], f16, tag="exp", name="exp", bufs=18)
            nc.scalar.activation(ex[:], sc_ps[:], Exp,
                                 bias=mask_s[:, kc:kc + 1], scale=0.125)
            pend.append((ci, kc, ex))

        def pop_ctx():
            """Emit the ctx matmuls for the oldest pending exp (ready work --
            its exp completed ~10 steps ago). start=False always: a matmul
            start zeroes its ENTIRE PSUM bank, so the 4 interleaved per-qt
            groups sharing a bank are zeroed once up front by a
            zero-stationary matmul. On the last kc, normalize straight out
            of PSUM into asm (DVE), freeing the combo's 2 ctx banks."""
            ci, kc, ex = pend.pop(0)
            hp, qb4 = combos[ci]
            if ci not in ctx_open:
                ctx_open[ci] = [
                    psC.tile([128, 4, 65], f32, tag="ctx",
                             name=f"ctx{ci}_{j}") for j in range(2)]
                for t in ctx_open[ci]:
                    nc.tensor.matmul(
                        t.rearrange("p a b -> p (a b)"), z260[:, 0:128],
                        z260[:], start=True, stop=False,
                        skip_group_check=True)
            ctxh = ctx_open[ci]
            for j in range(2):
                h = hp * 2 + j
                for qt in range(4):
                    nc.tensor.matmul(
                        ctxh[j][:, qt, :],
                        ex[:, j * 512 + qt * 128:j * 512 + (qt + 1) * 128],
                        vt[kc][:, h * 65:(h + 1) * 65],
                        start=False, stop=(kc == KC - 1),
                        skip_group_check=True)
            if kc == KC - 1:
                last = ci == len(combos) - 1
                for j in range(2):
                    h = hp * 2 + j
                    rc4 = work.tile([128, 4], f32, tag="rc", name="rc",
                                    bufs=2)
                    nc.vector.reciprocal(rc4[:], ctx_open[ci][j][:, :, 64])
                    for qt in range(4):
                        dst = asm[qb4 * 4 + qt][:, h * 64:(h + 1) * 64]
                        src = ctx_open[ci][j][:, qt, 0:64]
                        if last and j == 1:
                            # drain: ACT is idle, split normalize across
                            # engines so it doesn't gate the final tails
                            nc.scalar.activation(dst, src, Ident,
                                                 scale=rc4[:, qt:qt + 1])
                        else:
                            nc.vector.tensor_scalar_mul(dst, src,
                                                        rc4[:, qt:qt + 1])
                del ctx_open[ci]

        def tail_t2p(gqt, ps, ptag, act):
            t2p = ps.tile([128, 256], f16, tag=ptag, name="t2p")
            for cc in range(2):
                nc.tensor.transpose(
                    t2p[:, cc * 128:(cc + 1) * 128],
                    asm[gqt][:, cc * 128:(cc + 1) * 128], id128[:])
            ctn = work.tile([128, 256], f16, tag="ctn", name="ctn", bufs=4)
            (nc.scalar.copy if act else nc.vector.tensor_copy)(ctn[:], t2p[:])
            return ctn

        def tail_op(gqt, ctn, fj, ps, ptag, cp):
            op = ps.tile([128, 512], f32, tag=ptag, name="op")
            for cc in range(2):
                nc.tensor.matmul(
                    op[:], ctn[:, cc * 128:(cc + 1) * 128],
                    wo_a[:, cc * H + fj * 512:cc * H + (fj + 1) * 512],
                    start=(cc == 0), stop=(cc == 1))
            ob = work.tile([128, 512], f32, tag="ob", name="ob", bufs=4)
            cp(ob[:], op[:])
            nc.sync.dma_start(
                out_d[gqt * 128:(gqt + 1) * 128, fj * 512:(fj + 1) * 512],
                ob[:])

        def tail(qb4, qts=range(4), act=False, ob_eng=None):
            if not act:
                for qt in qts:
                    gqt = qb4 * 4 + qt
                    ctn = tail_t2p(gqt, psD, "tp", False)
                    for fj in range(2):
                        tail_op(gqt, ctn, fj, psD, "op",
                                ob_eng or nc.vector.tensor_copy)
                return
            # final drain (scores done): software-pipeline the 4 qt chains
            # through the freed psA banks (bufs=2), alternating the PSUM->SBUF
            # copies between ACT and DVE so neither engine serializes it.
            gqts = [qb4 * 4 + qt for qt in qts]
            ctns = {}
            for i, gqt in enumerate(gqts):
                ctns[gqt] = tail_t2p(gqt, psA, "sc", i % 2 == 0)
                if i >= 1:
                    g = gqts[i - 1]
                    for fj in range(2):
                        tail_op(g, ctns[g], fj, psA, "sc",
                                nc.scalar.copy if (fj + i) % 2 else
                                nc.vector.tensor_copy)
            g = gqts[-1]
            for fj in range(2):
                tail_op(g, ctns[g], fj, psA, "sc",
                        nc.scalar.copy if fj else nc.vector.tensor_copy)

        # ---- flat-stream schedule ----
        # One global stream of 128 (combo, kc) score+exp steps keeps ACT
        # (the bottleneck engine) fed continuously; ctx matmuls trail ~10-12
        # steps behind via pend, with a 4-step breathing gap at each combo
        # boundary so the previous combo's normalize (DVE) frees its PSUM
        # banks before the next combo's first ctx write needs them.
        # Projections/tails are deadline-placed PE fillers. Per-step order is
        # pops -> fillers -> scores: the scores matmul self-throttles on the
        # psA rotation (waits exp(step-2)), so ready work must precede it in
        # the in-order PE stream.
        NSTEP = 8 * KC
        POP = {}
        for fj in range(NSTEP):
            c, kc = divmod(fj, KC)
            s = 16 * c + 12 + (kc * 11 + 8) // 16
            POP[s] = POP.get(s, 0) + 1

        F = {}

        def at(s, fn):
            F.setdefault(s, []).append(fn)

        def q_item(pi, sc):
            return lambda: qk_proj(wq_a, bq_s, qT, q8r, pi, sc,
                                   regroup=False)

        def k_item(pi, sc):
            return lambda: qk_proj(wk_a, bk_s, kT, k8r, pi, sc,
                                   regroup=False)

        def rg_item(dst, r8, pi, sc):
            def go():
                for g in range(4):
                    nc.gpsimd.dma_start(
                        r8[pi][:, g, sc * SC:(sc + 1) * SC],
                        dst[pi][g * 32:(g + 1) * 32, sc * SC:(sc + 1) * SC])
            return go

        def qrg(pi, sc):
            return rg_item(qT, q8r, pi, sc)

        def krg(pi, sc):
            return rg_item(kT, k8r, pi, sc)

        # hp0 projections feed combo 0 (fp16, direct from the f16 tiles) at
        # steps 4/8/12; the cast-regroups run on the serial Pool queue in
        # fp8-deadline order (first fp8 consumer: combo 1 from step 16).
        at(1, k_item(0, 1)), at(3, q_item(0, 1))
        at(5, k_item(0, 2)), at(9, k_item(0, 3))
        at(2, krg(0, 1)), at(4, qrg(0, 1)), at(7, krg(0, 2))
        at(11, krg(0, 3))
        for kc in range(KC):       # v_kc feeds the ctx pop at step 12+...
            at(6 + kc, lambda kc=kc: v_proj(kc))
        at(19, q_item(0, 2)), at(21, qrg(0, 2))
        at(35, q_item(0, 3)), at(37, qrg(0, 3))
        at(49, q_item(1, 0)), at(50, qrg(1, 0))
        at(53, k_item(1, 0)), at(54, krg(1, 0))
        at(57, k_item(1, 1)), at(58, krg(1, 1))
        at(61, q_item(1, 1)), at(62, qrg(1, 1))
        at(65, k_item(1, 2)), at(66, krg(1, 2))
        at(69, k_item(1, 3)), at(70, krg(1, 3))
        at(73, q_item(1, 2)), at(74, qrg(1, 2))
        at(77, q_item(1, 3)), at(78, qrg(1, 3))
        # tails for q-block X need BOTH (0,X) and (1,X) normalized; (1,X)
        # normalizes at step 16*(4+X)+22.
        # the last in-stream tails (2,*) sit as early as their normalize
        # dependency (step 118) allows, so their DVE out-copies drain before
        # the final combo's normalize needs DVE.
        for qt in range(4):
            at(89 + 3 * qt, lambda qt=qt: tail(0, [qt]))
            at(105 + 3 * qt, lambda qt=qt: tail(1, [qt]))
            at(119 + qt, lambda qt=qt: tail(2, [qt]))

        # q chunk (0, sc0) feeds only combo 0, which runs fp16 -> no regroup
        qk_proj(wq_a, bq_s, qT, q8r, 0, 0, act=True, regroup=False)
        qk_proj(wk_a, bk_s, kT, k8r, 0, 0, act=True)
        # per step: pops (frees PSUM, keeps normalize on schedule), fillers,
        # scores. In the last stretch the fillers (tails) move after scores:
        # there they would otherwise delay the final exps, which nothing can
        # hide anymore.
        for s in range(NSTEP):
            for _ in range(POP.get(s, 0)):
                pop_ctx()
            if s < 118:
                for fn in F.get(s, ()):
                    fn()
                emit_scores_exp(*divmod(s, KC))
            else:
                emit_scores_exp(*divmod(s, KC))
                for fn in F.get(s, ()):
                    fn()
        while pend:
            pop_ctx()
        tail(3, act=True)

        work.release()
        psD.release()
        psC.release()
        psA.release()
        pers.release()

    nc.compile()
    return nc


def _get_nc():
    if "nc" not in _CACHE:
        _CACHE["nc"] = _build()
    return _CACHE["nc"]


def kernel(hidden_states, attention_mask, Wq, bq, Wk, bk, Wv, bv, Wo, bo):
    from concourse.bass_utils import run_bass_kernel_spmd

    hidden_states = np.asarray(hidden_states, np.float32)
    attention_mask = np.asarray(attention_mask, np.float32)
    Wq, Wk, Wv, Wo = (np.asarray(a, np.float32) for a in (Wq, Wk, Wv, Wo))
    bq, bk, bv, bo = (np.asarray(a, np.float32) for a in (bq, bk, bv, bo))

    nc = _get_nc()
    in_maps = []
    xTb = [np.ascontiguousarray(hidden_states[b].T).astype(np.float16)
           for b in range(B)]
    maskb = [np.ascontiguousarray(attention_mask[b, 0, 0, :])
             for b in range(B)]
    for c in range(NCORES):
        b, g = c // HPC, c % HPC
        cs = slice(g * COLS, (g + 1) * COLS)
        in_maps.append({
            "xT": xTb[b],
            "wq": np.ascontiguousarray(Wq[:, cs]).astype(np.float16),
            "wk": np.ascontiguousarray(Wk[:, cs]).astype(np.float16),
            "wv": np.ascontiguousarray(Wv[:, cs]).astype(np.float16),
            "wo": np.ascontiguousarray(Wo[cs, :]).astype(np.float16),
            "bq": np.ascontiguousarray(bq[cs]),
            "bk": np.ascontiguousarray(bk[cs]),
            "mask": maskb[b],
        })

    trace = bool(os.environ.get("KERNEL_TRACE"))
    kw = {}
    if trace:
        kw = dict(trace=True, tmpdir=os.environ.get("KERNEL_TRACE_DIR"))
    res = run_bass_kernel_spmd(nc, in_maps, list(range(NCORES)), **kw)
    _CACHE["last_result"] = res

    out = np.zeros((B, S, H), np.float32)
    for c in range(NCORES):
        out[c // HPC] += res.results[c]["out"]
    out += bv @ Wo + bo
    return out


# revision 51
# speedup vs baseline: 1.0011x; 1.0011x over previous
"""Multi-head attention (B=2, S=2048, H=1024, NH=16 heads of 64) on 8 trn2
NeuronCores, tensor-parallel over heads with batch parallelism on top.

Sharding: core c handles batch b=c//4 and head-group g=c%4 (4 heads, 256 of
the 1024 hidden cols). Each core computes the partial output
ctx_g @ Wo[g_rows, :]; the host sums the 4 partials per batch and adds the
closed-form bias terms (bv @ Wo + bo; bq/bk are applied on-device).

Device math (per core), fp16 matmul operands, fp32 PSUM accumulation:

  qT/kT = Wq_g^T x_b^T (+bias/partition)  [2 head-pair tiles of 128 x 2048]
  v     = x_b Wv_g                        [16 tiles of 128 x (4*65)]; col 64
                                          of each head block = 1.0 (softmax
                                          denominator comes out of the ctx
                                          matmul for free)
  scoresT[k,q] = kT.T qT                  (PE; head pairs packed as
                                           [h0 512q | h1 512q] per 2-bank
                                           PSUM tile)
  expT = exp(0.125*scoresT + mask[k])     (one ACT op: scale+mask+exp; no max
                                           subtraction needed -- |scores|<~4)
  ctx[q,c] += expT_chunk^T v_aug          (PE; exp chunk [128k,128q] is the
                                           STATIONARY operand, v_aug [128k,65]
                                           moves -> 65 cycles/tile in the cost
                                           model instead of 512, and ctx lands
                                           in [q,c] orientation with D[q] in
                                           col 64 as a per-partition column)
  normalize: reciprocal of ctx[:,:,64] + per-partition tensor_scalar_mul
             straight into asm[q,c] (no transposes, no PSUM->SBUF ctx copy)
  out[q,:] = asm_n @ Wo_g                 (PE, via one transpose back to [c,q])

Precision: scores run fp8e4 (q/k cast+regrouped by gpsimd DMAs into the
DoubleRow 2x32 split-contraction layout, 0.5 cycles/row on the PE) except for
the first combo, which reads the f16 projections directly so the regroup
latency stays off the startup critical path. Everything else is fp16.

Schedule: one flat stream of 128 (combo, kc) score+exp steps keeps ACT (the
exp engine, ~137us busy, the bottleneck) fed continuously; ctx matmuls trail
~10 steps behind via a pop queue with a 6-step breathing gap at each combo
boundary (the previous combo's DVE normalize must free its 2 PSUM ctx banks
before the next combo's first ctx write). Projections, fp8 regroups and
output-projection tails are deadline-placed PE/Pool fillers. Combo order is
hp0-first to minimize projection front-load. Cost model: ~168us (PE 117us,
ACT 140us busy).
"""

import os
import sys

sys.path.insert(0, "/opt/trn_rl_repo")

import numpy as np

B, S, H, NH, HD = 2, 2048, 1024, 16, 64
NCORES = 8
HPC = 4          # heads per core
COLS = HPC * HD  # 256
KC = S // 128    # 16 k chunks
QB = 1024        # q block width (2 heads x 512 q)
NQT = S // 128   # 16 global q tiles
SC = 512         # seq chunk for projections

_CACHE = {}


def _build():
    import concourse.mybir as mybir
    import concourse.tile as tile
    from concourse import bacc
    from concourse.masks import make_identity

    f32 = mybir.dt.float32
    f16 = mybir.dt.float16
    f8 = mybir.dt.float8e4
    DR = mybir.MatmulPerfMode.DoubleRow
    Exp = mybir.ActivationFunctionType.Exp

    nc = bacc.Bacc("TRN2", target_bir_lowering=False, debug=False,
                   num_devices=NCORES)

    xT_d = nc.dram_tensor("xT", [H, S], f16, kind="ExternalInput").ap()
    wq_d = nc.dram_tensor("wq", [H, COLS], f16, kind="ExternalInput").ap()
    wk_d = nc.dram_tensor("wk", [H, COLS], f16, kind="ExternalInput").ap()
    wv_d = nc.dram_tensor("wv", [H, COLS], f16, kind="ExternalInput").ap()
    wo_d = nc.dram_tensor("wo", [COLS, H], f16, kind="ExternalInput").ap()
    bq_d = nc.dram_tensor("bq", [COLS], f32, kind="ExternalInput").ap()
    bk_d = nc.dram_tensor("bk", [COLS], f32, kind="ExternalInput").ap()
    mask_d = nc.dram_tensor("mask", [S], f32, kind="ExternalInput").ap()
    out_d = nc.dram_tensor("out", [S, H], f32, kind="ExternalOutput").ap()

    with tile.TileContext(nc) as tc:
        pers = tc.alloc_tile_pool(name="pers", bufs=1)
        # PSUM budget (8 banks of 2KB):
        #   psA: scores [128,1024] f32 (2 banks) x2 bufs           = 4
        #   psC: ctx accumulators [128,4,65] f32 (1 bank) x2 bufs  = 2
        #   psD: op [128,512] f32 (1) + t2p [128,256] f16 (1)      = 2
        psA = tc.alloc_tile_pool(name="psA", bufs=2, space="PSUM")
        psC = tc.alloc_tile_pool(name="psC", bufs=2, space="PSUM")
        psD = tc.alloc_tile_pool(name="psD", bufs=1, space="PSUM")
        work = tc.alloc_tile_pool(name="work", bufs=3)

        # q/k projected to f16; gpsimd cast-regroup DMAs produce the fp8
        # DoubleRow layout (head j's 64 d-dims as 2 k-tiles x 32 partitions)
        # for every combo except the first, whose scores run fp16 directly
        # so the regroup latency stays off the startup critical path.
        qT = [pers.tile([128, S], f16, tag=f"qT{i}", name=f"qT{i}")
              for i in range(2)]
        kT = [pers.tile([128, S], f16, tag=f"kT{i}", name=f"kT{i}")
              for i in range(2)]
        q8r = [pers.tile([32, 4, S], f8, tag=f"q8r{i}", name=f"q8r{i}")
               for i in range(2)]
        k8r = [pers.tile([32, 4, S], f8, tag=f"k8r{i}", name=f"k8r{i}")
               for i in range(2)]
        vt = [pers.tile([128, HPC * 65], f16, tag=f"v{i}", name=f"v{i}")
              for i in range(KC)]
        asm = [pers.tile([128, COLS], f16, tag=f"asm{i}", name=f"asm{i}")
               for i in range(NQT)]
        xt4 = [pers.tile([128, 2 * S], f16, tag=f"xt4{i}", name=f"xt4{i}")
               for i in range(4)]
        wq_a = pers.tile([128, 2048], f16, tag="wq", name="wq_a")
        wk_a = pers.tile([128, 2048], f16, tag="wk", name="wk_a")
        wv_a = pers.tile([128, 2048], f16, tag="wv", name="wv_a")
        wo_a = pers.tile([128, 2048], f16, tag="wo", name="wo_a")

        def xT(hc):
            """View of H-chunk hc of x^T: [128, S] slice of a packed tile."""
            return xt4[hc // 2][:, (hc % 2) * S:(hc % 2) * S + S]
        bq_s = pers.tile([128, 2], f32, tag="bq", name="bq_s")
        bk_s = pers.tile([128, 2], f32, tag="bk", name="bk_s")
        mask_s = pers.tile([128, KC], f32, tag="mask", name="mask_s")
        id128 = pers.tile([128, 128], f16, tag="id128", name="id128")
        z260 = pers.tile([128, 260], f16, tag="z260", name="z260")

        warm = pers.tile([1, 1], f32, tag="warm", name="warm")
        nc.gpsimd.memset(warm[:], 0.0)
        nc.scalar.activation(warm[:], warm[:], Exp)
        make_identity(nc, id128[:])
        nc.vector.memset(z260[:], 0.0)

        # Few large DMAs on one HWDGE queue (dispatch is ~650ns/DMA, so
        # batch aggressively), ordered so the projection pipeline starts as
        # early as possible (queue order = arrival order).
        def xt4_pair(t, lo, hi):
            out = xt4[t].rearrange("p (c s) -> p c s", c=2)[:, :, lo:hi]
            in_ = xT_d[t * 256:(t + 1) * 256, lo:hi].rearrange(
                "(c p) s -> p c s", p=128)
            nc.sync.dma_start(out, in_)

        def w_half(w_a, w_d, lo):
            nc.sync.dma_start(
                w_a.rearrange("p (c n) -> p c n", c=8)[:, 4 * lo:4 * lo + 4],
                w_d[512 * lo:512 * lo + 512].rearrange(
                    "(c p) n -> p c n", p=128))

        w_half(wq_a, wq_d, 0)
        for t in range(4):
            xt4_pair(t, 0, SC)
        w_half(wq_a, wq_d, 1)
        w_half(wk_a, wk_d, 0)
        w_half(wk_a, wk_d, 1)
        nc.sync.dma_start(bq_s[:], bq_d.rearrange("(a p) -> p a", p=128))
        nc.sync.dma_start(bk_s[:], bk_d.rearrange("(a p) -> p a", p=128))
        nc.sync.dma_start(mask_s[:], mask_d.rearrange("(a p) -> p a", p=128))
        for t in range(4):
            xt4_pair(t, SC, 2 * SC)
        nc.sync.dma_start(wv_a.rearrange("p (c n) -> p c n", c=8),
                          wv_d.rearrange("(c p) n -> p c n", p=128))
        for t in range(4):
            xt4_pair(t, 2 * SC, 3 * SC)
        for t in range(4):
            xt4_pair(t, 3 * SC, S)
        nc.sync.dma_start(wo_a.rearrange("p (c n) -> p c n", c=2),
                          wo_d.rearrange("(c p) n -> p c n", p=128))

        Ident = mybir.ActivationFunctionType.Identity

        def qk_proj(w_a, b_s, dst, r8, pi, sc, act=False, regroup=True,
                    hcs=range(8), ps=None):
            # psD slot (not psA): projections must not perturb the scores
            # double-buffer rotation, or ACT stalls an extra exp every time.
            if ps is None:
                ps = psD.tile([128, SC], f32, tag="op", name="pps")
            for hc in hcs:
                nc.tensor.matmul(
                    ps[:], w_a[:, hc * COLS + pi * 128:hc * COLS + pi * 128 + 128],
                    xT(hc)[:, sc * SC:(sc + 1) * SC],
                    start=(hc == 0), stop=(hc == 7))
            if hcs[-1] != 7:
                return ps
            if act:
                # phase-1: ACT has idle slots there
                nc.scalar.activation(dst[pi][:, sc * SC:(sc + 1) * SC], ps[:],
                                     Ident, bias=b_s[:, pi:pi + 1])
            else:
                nc.vector.tensor_scalar_add(dst[pi][:, sc * SC:(sc + 1) * SC],
                                            ps[:], b_s[:, pi:pi + 1])
            # cast-regroup this 512-q chunk into the fp8 DoubleRow
            # (p, (j,t)) layout; a DMA source AP cannot stride across
            # partitions in a free dim, so one plain partition-range DMA per
            # 32-partition group. Via GPSIMD software DGE (casts f16->f8;
            # waiting DMAs on the SP/ACT HWDGE rings would block those
            # engines' in-order sequencers).
            if regroup:
                for g in range(4):
                    nc.gpsimd.dma_start(
                        r8[pi][:, g, sc * SC:(sc + 1) * SC],
                        dst[pi][g * 32:(g + 1) * 32, sc * SC:(sc + 1) * SC])

        def v_proj(st):
            ps = psD.tile([128, COLS], f32, tag="op", name="vps")
            for hc in range(8):
                nc.tensor.matmul(ps[:], xT(hc)[:, st * 128:(st + 1) * 128],
                                 wv_a[:, hc * COLS:(hc + 1) * COLS],
                                 start=(hc == 0), stop=(hc == 7))
            nc.vector.memset(vt[st][:], 1.0)
            nc.vector.tensor_copy(
                vt[st].rearrange("p (h c) -> p h c", c=65)[:, :, 0:64],
                ps[:].rearrange("p (h c) -> p h c", c=64))

        # ---- attention machinery ----
        combos = [(0, 0), (0, 1), (0, 2), (0, 3),
                  (1, 0), (1, 1), (1, 2), (1, 3)]
        ctx_open = {}   # combo idx -> [h0_tile, h1_tile] PSUM accumulators
        pend = []       # FIFO of (combo_idx, kc, ex) awaiting ctx emission

        def emit_scores_exp(ci, kc):
            """scores+exp for one (combo, kc): 2 score matmuls into a
            double-buffered PSUM tile [128, 1024] = [h0 512q | h1 512q],
            one ACT exp op. The psA rotation self-throttles PE ~2 steps
            behind ACT."""
            hp, qb4 = combos[ci]
            qs = qb4 * 512
            sc_ps = psA.tile([128, QB], f32, tag="sc", name="sc_ps")
            for j in range(2):
                if ci == 0:
                    # first combo: fp16 from the f16 tiles directly -- its
                    # cast-regroups would otherwise gate the score stream
                    # during startup
                    nc.tensor.matmul(
                        sc_ps[:, j * 512:(j + 1) * 512],
                        kT[hp][j * 64:j * 64 + 64, kc * 128:(kc + 1) * 128],
                        qT[hp][j * 64:j * 64 + 64, qs:qs + 512],
                        start=True, stop=True)
                else:
                    nc.tensor.matmul(
                        sc_ps[:, j * 512:(j + 1) * 512],
                        k8r[hp][:, 2 * j:2 * j + 2, kc * 128:(kc + 1) * 128],
                        q8r[hp][:, 2 * j:2 * j + 2, qs:qs + 512],
                        start=True, stop=True, perf_mode=DR)
            ex = work.tile([128, QB], f16, tag="exp", name="exp", bufs=18)
            nc.scalar.activation(ex[:], sc_ps[:], Exp,
                                 bias=mask_s[:, kc:kc + 1], scale=0.125)
            pend.append((ci, kc, ex))

        def pop_ctx():
            """Emit the ctx matmuls for the oldest pending exp (ready work --
            its exp completed ~10 steps ago). start=False always: a matmul
            start zeroes its ENTIRE PSUM bank, so the 4 interleaved per-qt
            groups sharing a bank are zeroed once up front by a
            zero-stationary matmul. On the last kc, normalize straight out
            of PSUM into asm (DVE), freeing the combo's 2 ctx banks."""
            ci, kc, ex = pend.pop(0)
            hp, qb4 = combos[ci]
            if ci not in ctx_open:
                ctx_open[ci] = [
                    psC.tile([128, 4, 65], f32, tag="ctx",
                             name=f"ctx{ci}_{j}") for j in range(2)]
                for t in ctx_open[ci]:
                    nc.tensor.matmul(
                        t.rearrange("p a b -> p (a b)"), z260[:, 0:128],
                        z260[:], start=True, stop=False,
                        skip_group_check=True)
            ctxh = ctx_open[ci]
            for j in range(2):
                h = hp * 2 + j
                for qt in range(4):
                    nc.tensor.matmul(
                        ctxh[j][:, qt, :],
                        ex[:, j * 512 + qt * 128:j * 512 + (qt + 1) * 128],
                        vt[kc][:, h * 65:(h + 1) * 65],
                        start=False, stop=(kc == KC - 1),
                        skip_group_check=True)
            if kc == KC - 1:
                last = ci == len(combos) - 1
                for j in range(2):
                    h = hp * 2 + j
                    rc4 = work.tile([128, 4], f32, tag="rc", name="rc",
                                    bufs=2)
                    nc.vector.reciprocal(rc4[:], ctx_open[ci][j][:, :, 64])
                    for qt in range(4):
                        dst = asm[qb4 * 4 + qt][:, h * 64:(h + 1) * 64]
                        src = ctx_open[ci][j][:, qt, 0:64]
                        if last and j == 1:
                            # drain: ACT is idle, split normalize across
                            # engines so it doesn't gate the final tails
                            nc.scalar.activation(dst, src, Ident,
                                                 scale=rc4[:, qt:qt + 1])
                        else:
                            nc.vector.tensor_scalar_mul(dst, src,
                                                        rc4[:, qt:qt + 1])
                del ctx_open[ci]

        def tail_t2p(gqt, ps, ptag, act):
            t2p = ps.tile([128, 256], f16, tag=ptag, name="t2p")
            for cc in range(2):
                nc.tensor.transpose(
                    t2p[:, cc * 128:(cc + 1) * 128],
                    asm[gqt][:, cc * 128:(cc + 1) * 128], id128[:])
            ctn = work.tile([128, 256], f16, tag="ctn", name="ctn", bufs=4)
            (nc.scalar.copy if act else nc.vector.tensor_copy)(ctn[:], t2p[:])
            return ctn

        def tail_op(gqt, ctn, fj, ps, ptag, cp):
            op = ps.tile([128, 512], f32, tag=ptag, name="op")
            for cc in range(2):
                nc.tensor.matmul(
                    op[:], ctn[:, cc * 128:(cc + 1) * 128],
                    wo_a[:, cc * H + fj * 512:cc * H + (fj + 1) * 512],
                    start=(cc == 0), stop=(cc == 1))
            ob = work.tile([128, 512], f32, tag="ob", name="ob", bufs=4)
            cp(ob[:], op[:])
            nc.sync.dma_start(
                out_d[gqt * 128:(gqt + 1) * 128, fj * 512:(fj + 1) * 512],
                ob[:])

        def tail(qb4, qts=range(4), act=False):
            if not act:
                for qt in qts:
                    gqt = qb4 * 4 + qt
                    ctn = tail_t2p(gqt, psD, "tp", False)
                    for fj in range(2):
                        tail_op(gqt, ctn, fj, psD, "op",
                                nc.vector.tensor_copy)
                return
            # final drain (scores done): software-pipeline the 4 qt chains
            # through the freed psA banks (bufs=2), alternating the PSUM->SBUF
            # copies between ACT and DVE so neither engine serializes it.
            gqts = [qb4 * 4 + qt for qt in qts]
            ctns = {}
            for i, gqt in enumerate(gqts):
                ctns[gqt] = tail_t2p(gqt, psA, "sc", i % 2 == 0)
                if i >= 1:
                    g = gqts[i - 1]
                    for fj in range(2):
                        tail_op(g, ctns[g], fj, psA, "sc",
                                nc.scalar.copy if (fj + i) % 2 else
                                nc.vector.tensor_copy)
            g = gqts[-1]
            for fj in range(2):
                tail_op(g, ctns[g], fj, psA, "sc",
                        nc.scalar.copy if fj else nc.vector.tensor_copy)

        # ---- flat-stream schedule ----
        # One global stream of 128 (combo, kc) score+exp steps keeps ACT
        # (the bottleneck engine) fed continuously; ctx matmuls trail ~10-12
        # steps behind via pend, with a 4-step breathing gap at each combo
        # boundary so the previous combo's normalize (DVE) frees its PSUM
        # banks before the next combo's first ctx write needs them.
        # Projections/tails are deadline-placed PE fillers. Per-step order is
        # pops -> fillers -> scores: the scores matmul self-throttles on the
        # psA rotation (waits exp(step-2)), so ready work must precede it in
        # the in-order PE stream.
        NSTEP = 8 * KC
        POP = {}
        for fj in range(NSTEP):
            c, kc = divmod(fj, KC)
            s = 16 * c + 12 + (kc * 11 + 8) // 16
            POP[s] = POP.get(s, 0) + 1

        F = {}

        def at(s, fn):
            F.setdefault(s, []).append(fn)

        def q_item(pi, sc):
            return lambda: qk_proj(wq_a, bq_s, qT, q8r, pi, sc,
                                   regroup=False)

        def k_item(pi, sc):
            return lambda: qk_proj(wk_a, bk_s, kT, k8r, pi, sc,
                                   regroup=False)

        def rg_item(dst, r8, pi, sc):
            def go():
                for g in range(4):
                    nc.gpsimd.dma_start(
                        r8[pi][:, g, sc * SC:(sc + 1) * SC],
                        dst[pi][g * 32:(g + 1) * 32, sc * SC:(sc + 1) * SC])
            return go

        def qrg(pi, sc):
            return rg_item(qT, q8r, pi, sc)

        def krg(pi, sc):
            return rg_item(kT, k8r, pi, sc)

        # hp0 projections feed combo 0 (fp16, direct from the f16 tiles) at
        # steps 4/8/12; the cast-regroups run on the serial Pool queue in
        # fp8-deadline order (first fp8 consumer: combo 1 from step 16).
        at(1, k_item(0, 1)), at(3, q_item(0, 1))
        at(5, k_item(0, 2)), at(9, k_item(0, 3))
        at(2, krg(0, 1)), at(4, qrg(0, 1)), at(7, krg(0, 2))
        at(11, krg(0, 3))
        for kc in range(KC):       # v_kc feeds the ctx pop at step 12+...
            at(6 + kc, lambda kc=kc: v_proj(kc))
        at(19, q_item(0, 2)), at(21, qrg(0, 2))
        at(35, q_item(0, 3)), at(37, qrg(0, 3))
        at(49, q_item(1, 0)), at(50, qrg(1, 0))
        at(53, k_item(1, 0)), at(54, krg(1, 0))
        at(57, k_item(1, 1)), at(58, krg(1, 1))
        at(61, q_item(1, 1)), at(62, qrg(1, 1))
        at(65, k_item(1, 2)), at(66, krg(1, 2))
        at(69, k_item(1, 3)), at(70, krg(1, 3))
        at(73, q_item(1, 2)), at(74, qrg(1, 2))
        at(77, q_item(1, 3)), at(78, qrg(1, 3))
        # tails for q-block X need BOTH (0,X) and (1,X) normalized; (1,X)
        # normalizes at step 16*(4+X)+22.
        for qt in range(4):
            at(89 + 3 * qt, lambda qt=qt: tail(0, [qt]))
            at(105 + 3 * qt, lambda qt=qt: tail(1, [qt]))
            at(121 + 2 * qt, lambda qt=qt: tail(2, [qt]))

        # q chunk (0, sc0) feeds only combo 0, which runs fp16 -> no regroup
        qk_proj(wq_a, bq_s, qT, q8r, 0, 0, act=True, regroup=False)
        qk_proj(wk_a, bk_s, kT, k8r, 0, 0, act=True)
        # per step: pops (frees PSUM, keeps normalize on schedule), fillers,
        # scores. In the last stretch the fillers (tails) move after scores:
        # there they would otherwise delay the final exps, which nothing can
        # hide anymore.
        for s in range(NSTEP):
            for _ in range(POP.get(s, 0)):
                pop_ctx()
            if s < 118:
                for fn in F.get(s, ()):
                    fn()
                emit_scores_exp(*divmod(s, KC))
            else:
                emit_scores_exp(*divmod(s, KC))
                for fn in F.get(s, ()):
                    fn()
        while pend:
            pop_ctx()
        tail(3, act=True)

        work.release()
        psD.release()
        psC.release()
        psA.release()
        pers.release()

    nc.compile()
    return nc


def _get_nc():
    if "nc" not in _CACHE:
        _CACHE["nc"] = _build()
    return _CACHE["nc"]


def kernel(hidden_states, attention_mask, Wq, bq, Wk, bk, Wv, bv, Wo, bo):
    from concourse.bass_utils import run_bass_kernel_spmd

    hidden_states = np.asarray(hidden_states, np.float32)
    attention_mask = np.asarray(attention_mask, np.float32)
    Wq, Wk, Wv, Wo = (np.asarray(a, np.float32) for a in (Wq, Wk, Wv, Wo))
    bq, bk, bv, bo = (np.asarray(a, np.float32) for a in (bq, bk, bv, bo))

    nc = _get_nc()
    in_maps = []
    xTb = [np.ascontiguousarray(hidden_states[b].T).astype(np.float16)
           for b in range(B)]
    maskb = [np.ascontiguousarray(attention_mask[b, 0, 0, :])
             for b in range(B)]
    for c in range(NCORES):
        b, g = c // HPC, c % HPC
        cs = slice(g * COLS, (g + 1) * COLS)
        in_maps.append({
            "xT": xTb[b],
            "wq": np.ascontiguousarray(Wq[:, cs]).astype(np.float16),
            "wk": np.ascontiguousarray(Wk[:, cs]).astype(np.float16),
            "wv": np.ascontiguousarray(Wv[:, cs]).astype(np.float16),
            "wo": np.ascontiguousarray(Wo[cs, :]).astype(np.float16),
            "bq": np.ascontiguousarray(bq[cs]),
            "bk": np.ascontiguousarray(bk[cs]),
            "mask": maskb[b],
        })

    trace = bool(os.environ.get("KERNEL_TRACE"))
    kw = {}
    if trace:
        kw = dict(trace=True, tmpdir=os.environ.get("KERNEL_TRACE_DIR"))
    res = run_bass_kernel_spmd(nc, in_maps, list(range(NCORES)), **kw)
    _CACHE["last_result"] = res

    out = np.zeros((B, S, H), np.float32)
    for c in range(NCORES):
        out[c // HPC] += res.results[c]["out"]
    out += bv @ Wo + bo
    return out
